# revision 45
# baseline (speedup 1.0000x reference)
"""Trainium2 Bass kernel for ContractiveInvertibleGNN feed-forward.

Math (reference, with group_mask == I_32):
  out[b,i] = f_i( sum_j W_adj[j,i] * g_j(X[b,j]) )
where g_j: R -> R^32 and f_i: R^32 -> R are slices of two shared MLPs
(64->128->128->32 with a residual middle block, LeakyReLU 0.01):
  g: H1 = lrelu(X[b,j]*U_j + C1_j); H2 = H1 + lrelu(H1@W2g + b2g)
     X_emb = H2 @ W3g + b3g
  f: Hf1 = lrelu(X_aggr@Wf1x + C2_i); Hf2 = Hf1 + lrelu(Hf1@Wf2 + bf2)
     out_i = Hf2 . V_i (+ bf3_i)
with per-node constants U_j = g_W1[j,:], C1_j = emb_j@g_W1[32:]+g_b1,
C2_i = emb_i@f_W1[32:]+f_b1 (+ (sum_j W_adj[j,i])*g_b3@f_W1[:32]),
V_i = f_W3[:,i].

Sharding: pure data-parallel over batch across 8 cores (2048 rows each).

Engine balance (the point of this implementation): activations are bf16 in
SBUF (psum stays f32). LeakyReLU work is split by column ranges across the
Activation engine (fused scale/bias lrelu), DVE (mult by alpha) and Pool
(max) so no single engine serializes the kernel. The f-phase residual
H2f = Hf1 + lrelu(...) is materialized with one DVE add so the final V-dot
needs 4 accumulating matmuls instead of 8. Matmuls run on bf16 operands
(1 cycle/row, same as f32r) which also halves DMA and removes the
f32->f32r staging copies of the earlier version.

On-chip layout (per core): node-major columns. g-phase runs per node j over
[128, 2048] tiles; X_emb assembled as Xe[(c,d), (j,t)] with c = batch
quarter stacked on partition groups; DVE transpose -> Xt[(c,j),(t,d)];
block-diag(W_adj) matmul aggregates over j; DVE transpose back ->
Xa[(c,d),(i,t)]; f-phase per node i with padded stationaries selecting
partition group c; final dot with V_i via a [128,32] stationary that also
routes batch quarter c to psum row c.
"""

import os
import sys

import numpy as np

for _p in ("/opt/trn_rl_repo", "/root/.axon_site/_ro/trn_rl_repo"):
    if os.path.isdir(_p) and _p not in sys.path:
        sys.path.insert(0, _p)

N = 32          # nodes
D = 32          # processed dim (== N, group_mask = I)
A = 128         # hidden width
B = 16384       # batch
NCORES = 8
BC = B // NCORES        # 2048 rows per core
CH = 512                # matmul free-dim chunk
NCH = BC // CH          # 4 chunks (partition-group stacking factor)
ALPHA = 0.01

# Column splits: how many of each per-node activation's 2048 columns run on
# the Activation engine. The rest: h1's tail runs as DVE z=x*u+c1, Pool
# n=(alpha-1)*min(z,0) (SBUF-sourced; GPSIMD cannot touch PSUM or do float
# max), then a DVE/Pool split of the final add. t2/tf tails run as DVE
# mult+max pairs or DVE-copy + Pool min/add chains. The f-phase residual
# Hf2 = hf1 + tf is materialized on DVE so V-dot needs only 4 accumulating
# matmuls per node, packed 8 nodes per PSUM tile.
H1A = 832               # h1 lrelu Act cols; rest DVE-z + Pool min + add
H1PD = 816              # of the h1 tail adds, cols done by DVE (rest Pool)
T2A = 1792              # t2 lrelu Act cols; rest DVE mult+max pair
TFA = 1216              # tf lrelu Act cols; rest DVE mult+max pair
NVP = 8                 # output nodes sharing one V-dot psum tile


def _build_program(zero_b2=True):
    from contextlib import ExitStack

    from concourse import bacc, mybir, tile

    f32 = mybir.dt.float32
    bf16 = mybir.dt.bfloat16
    LRELU = mybir.ActivationFunctionType.Lrelu
    ALU_MULT = mybir.AluOpType.mult
    ALU_ADD = mybir.AluOpType.add
    ALU_MAX = mybir.AluOpType.max
    ALU_MIN = mybir.AluOpType.min

    nc = bacc.Bacc("TRN2", target_bir_lowering=False, debug=False)

    def din(name, shape, dt):
        return nc.dram_tensor(
            name, list(shape), dt, kind="ExternalInput"
        ).ap()

    xt_d = din("XT", (N, BC), bf16)
    gw2_d = din("GW2", (A, A), bf16)
    fw2_d = din("FW2", (A, A), bf16)
    gw3p_d = din("GW3P", (A, NCH * A), bf16)   # col-block c: g_W3 at cols 32c..
    fw1p_d = din("FW1P", (A, NCH * A), bf16)   # row-block c: f_W1[:32] rows 32c..
    bd_d = din("BD", (A, A), bf16)             # kron(I4, W_adj)
    u_d = din("U", (A, N), f32)
    c1_d = din("C1", (A, N), f32)
    c2_d = din("C2", (A, N), f32)
    gb2_d = din("GB2", (A, 1), f32)
    fb2_d = din("FB2", (A, 1), f32)
    # V-dot stationary: slice (i,c) = cols [(i*NCH+c)*D, +D) with V_i at
    # column 4*(i%NVP)+c so NVP nodes' dots accumulate into one psum tile.
    vp_d = din("VP2", (A, N * NCH * D), bf16)
    out_d = nc.dram_tensor("OUT", [N, BC], f32, kind="ExternalOutput").ap()

    HCH = 2 * CH        # 1024: half of a node's batch columns
    T2AH = T2A // 2     # Act cols of t2 per half
    TFAH = TFA // 2     # Act cols of tf per half
    H1T = BC - H1A      # h1 tail cols (DVE/Pool path)

    with tile.TileContext(nc) as tc, ExitStack() as ctx:
        const = ctx.enter_context(tc.tile_pool(name="const", bufs=1))
        xep = ctx.enter_context(tc.tile_pool(name="xep", bufs=1))
        workp = ctx.enter_context(tc.tile_pool(name="work", bufs=2))
        scrp = ctx.enter_context(tc.tile_pool(name="scr", bufs=3))
        outp = ctx.enter_context(tc.tile_pool(name="outs", bufs=1))
        # PSUM: ppA 3x2 banks + ppB 1x1 + ppR 1x1 = 8 banks (the full file).
        # ppA needs 3 bufs: the f phase has FW1(i+1) and FW2(i) in flight
        # at once. ppR can be single-buffered because V-dot packs NVP nodes
        # per tile.
        ppA = ctx.enter_context(tc.tile_pool(name="ppA", bufs=3, space="PSUM"))
        ppB = ctx.enter_context(tc.tile_pool(name="ppB", bufs=1, space="PSUM"))
        ppR = ctx.enter_context(tc.tile_pool(name="ppR", bufs=1, space="PSUM"))

        def load_const(ap_dram, shape):
            t = const.tile(list(shape), ap_dram.dtype,
                           tag=f"c_{ap_dram.tensor.name}")
            nc.sync.dma_start(t[:, :], ap_dram)
            return t

        gw2_s = load_const(gw2_d, (A, A))
        fw2_s = load_const(fw2_d, (A, A))
        gw3p_s = load_const(gw3p_d, (A, NCH * A))
        fw1p_s = load_const(fw1p_d, (A, NCH * A))
        bd_s = load_const(bd_d, (A, A))
        u_s = load_const(u_d, (A, N))
        c1_s = load_const(c1_d, (A, N))
        c2_s = load_const(c2_d, (A, N))
        gb2_s = load_const(gb2_d, (A, 1))
        fb2_s = load_const(fb2_d, (A, 1))
        vp2_s = load_const(vp_d, (A, N * NCH * D))

        # Xe[(c,d), (j,t)] = X_emb[d, j, c*CH+t]
        xe = xep.tile([A, N * CH], bf16, tag="xe")

        # ---------------- g phase ----------------
        # Two-deep software pipeline. At iteration j the engines see only
        # ready work at their queue heads: the h1 "front" (Act lrelu, DVE z,
        # Pool min) is emitted two nodes ahead, its finishing adds one node
        # ahead, so nothing upstream of node j's matmuls ever waits.
        xbc_tiles = {}
        h1_front = {}
        h1_tiles = {}

        def emit_xbc(j):
            xbc = workp.tile([A, BC], bf16, tag="xbc", bufs=4)
            nc.sync.dma_start(
                xbc[:, :], xt_d[j : j + 1, :].partition_broadcast(A)
            )
            xbc_tiles[j] = xbc

        def emit_h1_front(j):
            xbc = xbc_tiles.pop(j)
            h1 = workp.tile([A, BC], bf16, tag="h1", bufs=3)
            nc.scalar.activation(
                h1[:, :H1A], xbc[:, :H1A], LRELU,
                bias=c1_s[:, j : j + 1], scale=u_s[:, j : j + 1], alpha=ALPHA,
            )
            # DVE: z = x*u + c1 (bf16). Pool (no PSUM access, no float max):
            # n = (alpha-1)*min(z,0). lrelu = z + n, add split DVE/Pool.
            zt = scrp.tile([A, H1T], bf16, tag="zt", bufs=3)
            mt = scrp.tile([A, H1T], bf16, tag="mt", bufs=3)
            nc.vector.tensor_scalar(zt[:, :], xbc[:, H1A:],
                                    u_s[:, j : j + 1], c1_s[:, j : j + 1],
                                    ALU_MULT, ALU_ADD)
            nc.gpsimd.tensor_scalar(mt[:, :], zt[:, :], 0.0, ALPHA - 1.0,
                                    ALU_MIN, ALU_MULT)
            h1_front[j] = (h1, zt, mt)

        def emit_h1_finish(j):
            h1, zt, mt = h1_front.pop(j)
            nc.vector.tensor_tensor(h1[:, H1A : H1A + H1PD], zt[:, :H1PD],
                                    mt[:, :H1PD], ALU_ADD)
            nc.gpsimd.tensor_tensor(h1[:, H1A + H1PD :], zt[:, H1PD:],
                                    mt[:, H1PD:], ALU_ADD)
            h1_tiles[j] = h1

        emit_xbc(0)
        emit_xbc(1)
        emit_xbc(2)
        emit_h1_front(0)
        emit_h1_front(1)
        emit_h1_finish(0)
        pm3_prev = {}
        for j in range(N):
            # xe copy for node j-1 first: it's a ready op for DVE's queue
            # head and frees pm3's single psum buffer before GW3P(j) asks.
            if j > 0:
                nc.vector.tensor_copy(xe[:, (j - 1) * CH : j * CH],
                                      pm3_prev.pop(j - 1)[:, :])
            # DMA three ahead, h1 front two ahead, finishing adds one ahead:
            # every queued op's inputs were produced at least one full
            # iteration earlier.
            if j + 3 < N:
                emit_xbc(j + 3)
            if j + 2 < N:
                emit_h1_front(j + 2)
            if j + 1 < N:
                emit_h1_finish(j + 1)
            h1 = h1_tiles.pop(j)
            t2 = workp.tile([A, BC], bf16, tag="t2")
            for h in range(2):  # halves of 1024 cols
                pa = ppA.tile([A, HCH], f32, tag="pA")
                for q in range(2):
                    sl = slice(h * HCH + q * CH, h * HCH + (q + 1) * CH)
                    nc.tensor.matmul(
                        pa[:, q * CH : (q + 1) * CH], gw2_s[:, :],
                        h1[:, sl], start=True, stop=True,
                    )
                off = h * HCH
                if T2AH:
                    nc.scalar.activation(
                        t2[:, off : off + T2AH], pa[:, :T2AH], LRELU,
                        bias=gb2_s[:, 0:1], alpha=ALPHA,
                    )
                pcols = HCH - T2AH
                if pcols:
                    # PSUM-sourced tail: DVE-only pair (GPSIMD can't read
                    # PSUM).
                    m2 = scrp.tile([A, pcols], bf16, tag="m2")
                    if zero_b2:
                        nc.vector.tensor_scalar(
                            m2[:, :], pa[:, T2AH:], ALPHA, None, ALU_MULT)
                        nc.vector.tensor_tensor(
                            t2[:, off + T2AH : off + HCH], pa[:, T2AH:],
                            m2[:, :], ALU_MAX)
                    else:
                        z2 = scrp.tile([A, pcols], bf16, tag="z2")
                        nc.vector.tensor_scalar(
                            z2[:, :], pa[:, T2AH:], gb2_s[:, 0:1], None,
                            ALU_ADD)
                        nc.gpsimd.tensor_scalar(
                            m2[:, :], z2[:, :], 0.0, ALPHA - 1.0,
                            ALU_MIN, ALU_MULT)
                        nc.gpsimd.tensor_tensor(
                            t2[:, off + T2AH : off + HCH], z2[:, :],
                            m2[:, :], ALU_ADD)
            # X_emb = g_W3^T @ (H1 + lrelu(.)) via 8 accumulating matmuls,
            # chunk c routed to psum rows 32c by the padded stationary.
            # h1 passes first: they are ready before t2 lands. The copy to
            # xe is deferred to the top of the next iteration.
            pm3 = ppB.tile([A, CH], f32, tag="pB")
            for c in range(NCH):
                nc.tensor.matmul(pm3[:, :], gw3p_s[:, c * A : (c + 1) * A],
                                 h1[:, c * CH : (c + 1) * CH],
                                 start=(c == 0), stop=False)
            for c in range(NCH):
                nc.tensor.matmul(pm3[:, :], gw3p_s[:, c * A : (c + 1) * A],
                                 t2[:, c * CH : (c + 1) * CH],
                                 start=False, stop=(c == NCH - 1))
            pm3_prev[j] = pm3
        nc.vector.tensor_copy(xe[:, (N - 1) * CH :],
                              pm3_prev.pop(N - 1)[:, :])

        # ---------------- aggregation ----------------
        # T1: Xe[(c,d),(j,t)] -> Xt[(c,j),(t,d)]
        xt3 = xe.rearrange("p (j t) -> p j t", j=N).transpose([0, 2, 1])
        xtile = xep.tile([A, CH * D], bf16, tag="xt")
        xto = xtile.rearrange("p (t d) -> p t d", d=D)
        TS = 8  # split into 8 ops for overlap
        tstep = CH // TS
        for s in range(TS):
            nc.vector.transpose(
                xto[:, s * tstep : (s + 1) * tstep, :],
                xt3[:, s * tstep : (s + 1) * tstep, :],
            )
        # agg windows + T2-back: psum[(c,i),(t16,d)] -> Xa[(c,d),(i,t)].
        # StreamTranspose can't convert dtypes: DVE transposes psum f32 into
        # a small rolling f32 buffer, then the (agg-idle) Pool engine does a
        # strided convert-copy into bf16 Xa.
        xa = xep.tile([A, N * CH], bf16, tag="xa")
        xa3 = xa.rearrange("p (i t) -> p i t", i=N).transpose([0, 2, 1])
        WT = HCH // D  # 32 t per window
        for w in range(CH // WT):  # 16 windows of 1024 cols
            pg = ppA.tile([A, HCH], f32, tag="pA", name=f"pg_{w}")
            for q in range(2):  # psum bank per matmul output
                nc.tensor.matmul(
                    pg[:, q * CH : (q + 1) * CH], bd_s[:, :],
                    xtile[:, w * HCH + q * CH : w * HCH + (q + 1) * CH],
                    start=True, stop=True,
                )
            xat = scrp.tile([A, HCH], f32, tag="xat", bufs=3)
            nc.vector.transpose(
                xat.rearrange("p (t i) -> p t i", i=D)[:, :, :],
                pg.rearrange("p (t d) -> p t d", d=D)[:, :, :],
            )
            nc.gpsimd.tensor_copy(
                xa3[:, w * WT : (w + 1) * WT, :],
                xat.rearrange("p (t i) -> p t i", i=D)[:, :, :],
            )

        # ---------------- f phase ----------------
        # Pipelined at HALF-node granularity (64 units u = 2*i + h): the f
        # dependency chain factors by halves (FW2 of half h reads only hf1
        # of half h; V-dot chunk c sits in one half), so with [A, 1024]
        # psum tiles and ppA bufs=3, three half-units can be in flight
        # where full nodes could not. Stages: S1 = FW1 + hf1 act,
        # S2 = FW2 + tf lrelu, S3 = hf2 add (Pool, half 0) + V-dot.
        # V-dot psum packing: NVP nodes share one [32, CH] psum tile; the
        # VP2 stationary routes node i chunk c to psum row 4*(i%NVP)+c.
        hf1_tiles = {}
        tf_half = {}
        vdot_state = {"pr": None}

        def emit_fs1(u):
            i, h = divmod(u, 2)
            if h == 0:
                hf1 = workp.tile([A, BC], bf16, tag="hf1", bufs=4,
                                 name=f"hf1_{i}")
                hf1_tiles[i] = hf1
            hf1 = hf1_tiles[i]
            rhs = xa[:, i * CH : (i + 1) * CH]
            pa = ppA.tile([A, HCH], f32, tag="pA", name=f"pf1_{u}")
            for q in range(2):
                c = h * 2 + q
                nc.tensor.matmul(
                    pa[:, q * CH : (q + 1) * CH],
                    fw1p_s[:, c * A : (c + 1) * A], rhs,
                    start=True, stop=True,
                )
            nc.scalar.activation(
                hf1[:, h * HCH : (h + 1) * HCH], pa[:, :], LRELU,
                bias=c2_s[:, i : i + 1], alpha=ALPHA,
            )

        def emit_fs2(u):
            i, h = divmod(u, 2)
            hf1 = hf1_tiles[i]
            tf = workp.tile([A, HCH], bf16, tag="tf", bufs=3,
                            name=f"tf_{u}")
            pa = ppA.tile([A, HCH], f32, tag="pA", name=f"pf2_{u}")
            for q in range(2):
                c = h * 2 + q
                nc.tensor.matmul(
                    pa[:, q * CH : (q + 1) * CH], fw2_s[:, :],
                    hf1[:, c * CH : (c + 1) * CH], start=True, stop=True,
                )
            if TFAH:
                nc.scalar.activation(
                    tf[:, :TFAH], pa[:, :TFAH], LRELU,
                    bias=fb2_s[:, 0:1], alpha=ALPHA,
                )
            pcols = HCH - TFAH
            if pcols:
                # PSUM-sourced tail: DVE-only pair (GPSIMD can't read PSUM).
                mf = scrp.tile([A, pcols], bf16, tag="mf")
                if zero_b2:
                    nc.vector.tensor_scalar(
                        mf[:, :], pa[:, TFAH:], ALPHA, None, ALU_MULT)
                    nc.vector.tensor_tensor(
                        tf[:, TFAH:], pa[:, TFAH:], mf[:, :], ALU_MAX)
                else:
                    zf = scrp.tile([A, pcols], bf16, tag="zf")
                    nc.vector.tensor_scalar(
                        zf[:, :], pa[:, TFAH:], fb2_s[:, 0:1], None,
                        ALU_ADD)
                    nc.gpsimd.tensor_scalar(
                        mf[:, :], zf[:, :], 0.0, ALPHA - 1.0,
                        ALU_MIN, ALU_MULT)
                    nc.gpsimd.tensor_tensor(
                        tf[:, TFAH:], zf[:, :], mf[:, :], ALU_ADD)
            tf_half[u] = tf

        def emit_fs3(u):
            i, h = divmod(u, 2)
            hf1 = hf1_tiles[i] if h == 0 else hf1_tiles.pop(i)
            tf = tf_half.pop(u)
            if u % (2 * NVP) == 0:
                vdot_state["pr"] = ppR.tile([D, CH], f32, tag="pR",
                                            name=f"pr_{u}")
            pr = vdot_state["pr"]
            for q in range(2):
                c = h * 2 + q
                base = (i * NCH + c) * D
                st = (i % NVP == 0 and c == 0)
                sp = (i % NVP == NVP - 1 and c == NCH - 1)
                if h == 0:
                    # Pool (idle in f) materializes hf2 for half 0
                    hp = scrp.tile([A, CH], bf16, tag="hf2c",
                                   name=f"hf2_{u}_{q}")
                    nc.gpsimd.tensor_tensor(
                        hp[:, :], hf1[:, c * CH : (c + 1) * CH],
                        tf[:, q * CH : (q + 1) * CH], ALU_ADD)
                    nc.tensor.matmul(pr[:, :], vp2_s[:, base : base + D],
                                     hp[:, :], start=st, stop=sp)
                else:
                    nc.tensor.matmul(pr[:, :], vp2_s[:, base : base + D],
                                     hf1[:, c * CH : (c + 1) * CH],
                                     start=st, stop=False)
                    nc.tensor.matmul(pr[:, :], vp2_s[:, base : base + D],
                                     tf[:, q * CH : (q + 1) * CH],
                                     start=False, stop=sp)
            if u % (2 * NVP) == 2 * NVP - 1:
                g0 = i - (NVP - 1)
                osb = outp.tile([NVP * NCH, CH], f32, tag="o")
                nc.vector.tensor_copy(osb[:, :], pr[: NVP * NCH, :])
                nc.sync.dma_start(
                    out_d[g0 : g0 + NVP, :].rearrange(
                        "o (c t) -> (o c) t", c=NCH),
                    osb[:, :],
                )

        NU = 2 * N
        emit_fs1(0)
        emit_fs2(0)
        emit_fs1(1)
        for u in range(NU):
            emit_fs3(u)
            if u + 2 < NU:
                emit_fs1(u + 2)
            if u + 1 < NU:
                emit_fs2(u + 1)

    nc.compile()
    return nc


_NC_CACHE = {}


def _get_program(zero_b2=True):
    if zero_b2 not in _NC_CACHE:
        _NC_CACHE[zero_b2] = _build_program(zero_b2)
    return _NC_CACHE[zero_b2]


def _bf16(x):
    import ml_dtypes
    return np.asarray(x, np.float32).astype(ml_dtypes.bfloat16)


def _host_consts(W, embeddings, g_W1, g_b1, g_W2, g_b2, g_W3, g_b3,
                 f_W1, f_b1, f_W2, f_b2, f_W3, f_b3):
    f = np.float32
    W_adj = (W * (1.0 - np.eye(N, dtype=f))).astype(f)
    U = np.ascontiguousarray(g_W1[:D].T, dtype=f)                    # [A, N]
    C1 = np.ascontiguousarray((embeddings @ g_W1[D:] + g_b1).T, f)   # [A, N]
    s = W_adj.sum(axis=0)                                            # [N]
    C2 = (embeddings @ f_W1[D:] + f_b1 + np.outer(s, g_b3 @ f_W1[:D]))
    C2 = np.ascontiguousarray(C2.T, dtype=f)                         # [A, N]
    GW3P = np.zeros((A, NCH * A), f)
    FW1P = np.zeros((A, NCH * A), f)
    for c in range(NCH):
        GW3P[:, c * A + c * D : c * A + (c + 1) * D] = g_W3
        FW1P[c * D : (c + 1) * D, c * A : (c + 1) * A] = f_W1[:D]
    BD = np.kron(np.eye(NCH, dtype=f), W_adj).astype(f)
    VP2 = np.zeros((A, N * NCH * D), f)
    for i in range(N):
        for c in range(NCH):
            VP2[:, (i * NCH + c) * D + NCH * (i % NVP) + c] = f_W3[:, i]
    return {
        "GW2": _bf16(g_W2),
        "FW2": _bf16(f_W2),
        "GW3P": _bf16(GW3P), "FW1P": _bf16(FW1P), "BD": _bf16(BD),
        "U": U, "C1": C1, "C2": C2,
        "GB2": np.ascontiguousarray(g_b2.reshape(A, 1), f),
        "FB2": np.ascontiguousarray(f_b2.reshape(A, 1), f),
        "VP2": _bf16(VP2),
    }


def _kernel_numpy(X, W, embeddings, g_W1, g_b1, g_W2, g_b2, g_W3, g_b3,
                  f_W1, f_b1, f_W2, f_b2, f_W3, f_b3, group_mask):
    # general fallback (non-identity group_mask)
    def lrelu(x):
        return np.where(x > 0, x, ALPHA * x)

    def mlp(x, W1, b1, W2, b2, W3, b3):
        h = lrelu(x @ W1 + b1)
        h = h + lrelu(h @ W2 + b2)
        return h @ W3 + b3

    n = W.shape[0]
    W_adj = W * (1.0 - np.eye(n, dtype=W.dtype))
    Xm = X[:, None, :] * group_mask
    E = np.broadcast_to(embeddings, (X.shape[0], n, embeddings.shape[1]))
    Xe = mlp(np.concatenate([Xm, E], 2), g_W1, g_b1, g_W2, g_b2, g_W3, g_b3)
    Xa = np.einsum("ji,bjd->bid", W_adj, Xe)
    Xr = mlp(np.concatenate([Xa, E], 2), f_W1, f_b1, f_W2, f_b2, f_W3, f_b3)
    return (Xr * group_mask).sum(axis=1).astype(np.float32)


def kernel(X, W, embeddings, g_W1, g_b1, g_W2, g_b2, g_W3, g_b3,
           f_W1, f_b1, f_W2, f_b2, f_W3, f_b3, group_mask, _run_kw=None):
    if not np.allclose(group_mask, np.eye(N, D, dtype=np.float32)):
        return _kernel_numpy(X, W, embeddings, g_W1, g_b1, g_W2, g_b2, g_W3,
                             g_b3, f_W1, f_b1, f_W2, f_b2, f_W3, f_b3,
                             group_mask)

    from concourse import bass_utils

    zero_b2 = not (np.any(g_b2) or np.any(f_b2))
    consts = _host_consts(W, embeddings, g_W1, g_b1, g_W2, g_b2, g_W3, g_b3,
                          f_W1, f_b1, f_W2, f_b2, f_W3, f_b3)
    XT = _bf16(np.asarray(X, np.float32).T)  # [N, B] bf16
    in_maps = []
    for k in range(NCORES):
        m = dict(consts)
        m["XT"] = np.ascontiguousarray(XT[:, k * BC : (k + 1) * BC])
        in_maps.append(m)

    nc = _get_program(zero_b2)
    res = bass_utils.run_bass_kernel_spmd(
        nc, in_maps, core_ids=list(range(NCORES)), **(_run_kw or {})
    )
    out = np.empty((B, D), np.float32)
    for k in range(NCORES):
        out[k * BC : (k + 1) * BC, :] = res.results[k]["OUT"].T
    out += f_b3.reshape(1, D).astype(np.float32)
    if _run_kw:
        kernel.last_results = res
    return out


# revision 46
# speedup vs baseline: 1.0020x; 1.0020x over previous
"""Trainium2 Bass kernel for ContractiveInvertibleGNN feed-forward.

Math (reference, with group_mask == I_32):
  out[b,i] = f_i( sum_j W_adj[j,i] * g_j(X[b,j]) )
where g_j: R -> R^32 and f_i: R^32 -> R are slices of two shared MLPs
(64->128->128->32 with a residual middle block, LeakyReLU 0.01):
  g: H1 = lrelu(X[b,j]*U_j + C1_j); H2 = H1 + lrelu(H1@W2g + b2g)
     X_emb = H2 @ W3g + b3g
  f: Hf1 = lrelu(X_aggr@Wf1x + C2_i); Hf2 = Hf1 + lrelu(Hf1@Wf2 + bf2)
     out_i = Hf2 . V_i (+ bf3_i)
with per-node constants U_j = g_W1[j,:], C1_j = emb_j@g_W1[32:]+g_b1,
C2_i = emb_i@f_W1[32:]+f_b1 (+ (sum_j W_adj[j,i])*g_b3@f_W1[:32]),
V_i = f_W3[:,i].

Sharding: pure data-parallel over batch across 8 cores (2048 rows each).

Engine balance (the point of this implementation): activations are bf16 in
SBUF (psum stays f32). LeakyReLU work is split by column ranges across the
Activation engine (fused scale/bias lrelu), DVE (alpha-mult + max pairs for
PSUM-sourced tails) and Pool (z + (alpha-1)*min(z,0) for SBUF-sourced
tails; GPSIMD cannot read PSUM or do float max). Matmuls run on bf16
operands (1 cycle/row, same as f32r, half the SBUF/DMA of f32). Both main
loops are software-pipelined two deep (g: h1 production runs two nodes
ahead; f: half-node units through FW1/hf1 -> FW2/tf -> hf2/V-dot stages)
so every engine's in-order queue holds ready work. V-dot output is packed
8 nodes per PSUM tile via a column-routed stationary.

On-chip layout (per core): node-major columns. g-phase runs per node j over
[128, 2048] tiles; X_emb assembled as Xe[(c,d), (j,t)] with c = batch
quarter stacked on partition groups; DVE transpose -> Xt[(c,j),(t,d)];
block-diag(W_adj) matmul aggregates over j; DVE transpose back ->
Xa[(c,d),(i,t)]; f-phase per node i with padded stationaries selecting
partition group c; final dot with V_i via a [128,32] stationary that also
routes batch quarter c to psum row c.
"""

import os
import sys

import numpy as np

for _p in ("/opt/trn_rl_repo", "/root/.axon_site/_ro/trn_rl_repo"):
    if os.path.isdir(_p) and _p not in sys.path:
        sys.path.insert(0, _p)

N = 32          # nodes
D = 32          # processed dim (== N, group_mask = I)
A = 128         # hidden width
B = 16384       # batch
NCORES = 8
BC = B // NCORES        # 2048 rows per core
CH = 512                # matmul free-dim chunk
NCH = BC // CH          # 4 chunks (partition-group stacking factor)
ALPHA = 0.01

# Column splits: how many of each per-node activation's 2048 columns run on
# the Activation engine. The rest: h1's tail runs as DVE z=x*u+c1, Pool
# n=(alpha-1)*min(z,0) (SBUF-sourced; GPSIMD cannot touch PSUM or do float
# max), then a DVE/Pool split of the final add. t2/tf tails run as DVE
# mult+max pairs. The f-phase residual Hf2 = hf1 + tf is materialized by
# Pool for half 0 (V-dot: 1 matmul/chunk) and left split for half 1
# (V-dot: 2 matmuls/chunk; PE has the slack).
H1A = 832               # h1 lrelu Act cols; rest DVE-z + Pool min + add
H1PD = 816              # of the h1 tail adds, cols done by DVE (rest Pool)
T2A = 1792              # t2 lrelu Act cols; rest DVE mult+max pair
TFA = 1216              # tf lrelu Act cols; rest DVE mult+max pair
NVP = 8                 # output nodes sharing one V-dot psum tile


def _build_program(zero_b2=True):
    from contextlib import ExitStack

    from concourse import bacc, mybir, tile

    f32 = mybir.dt.float32
    bf16 = mybir.dt.bfloat16
    LRELU = mybir.ActivationFunctionType.Lrelu
    ALU_MULT = mybir.AluOpType.mult
    ALU_ADD = mybir.AluOpType.add
    ALU_MAX = mybir.AluOpType.max
    ALU_MIN = mybir.AluOpType.min

    nc = bacc.Bacc("TRN2", target_bir_lowering=False, debug=False)

    def din(name, shape, dt):
        return nc.dram_tensor(
            name, list(shape), dt, kind="ExternalInput"
        ).ap()

    xt_d = din("XT", (N, BC), bf16)
    gw2_d = din("GW2", (A, A), bf16)
    fw2_d = din("FW2", (A, A), bf16)
    gw3p_d = din("GW3P", (A, NCH * A), bf16)   # col-block c: g_W3 at cols 32c..
    fw1p_d = din("FW1P", (A, NCH * A), bf16)   # row-block c: f_W1[:32] rows 32c..
    bd_d = din("BD", (A, A), bf16)             # kron(I4, W_adj)
    u_d = din("U", (A, N), f32)
    c1_d = din("C1", (A, N), f32)
    c2_d = din("C2", (A, N), f32)
    gb2_d = din("GB2", (A, 1), f32)
    fb2_d = din("FB2", (A, 1), f32)
    # V-dot stationary: slice (i,c) = cols [(i*NCH+c)*D, +D) with V_i at
    # column 4*(i%NVP)+c so NVP nodes' dots accumulate into one psum tile.
    vp_d = din("VP2", (A, N * NCH * D), bf16)
    out_d = nc.dram_tensor("OUT", [N, BC], f32, kind="ExternalOutput").ap()

    HCH = 2 * CH        # 1024: half of a node's batch columns
    T2AH = T2A // 2     # Act cols of t2 per half
    TFAH = TFA // 2     # Act cols of tf per half
    H1T = BC - H1A      # h1 tail cols (DVE/Pool path)

    with tile.TileContext(nc) as tc, ExitStack() as ctx:
        const = ctx.enter_context(tc.tile_pool(name="const", bufs=1))
        xep = ctx.enter_context(tc.tile_pool(name="xep", bufs=1))
        workp = ctx.enter_context(tc.tile_pool(name="work", bufs=2))
        scrp = ctx.enter_context(tc.tile_pool(name="scr", bufs=3))
        outp = ctx.enter_context(tc.tile_pool(name="outs", bufs=1))
        # PSUM: ppA 3x2 banks + ppB 1x1 + ppR 1x1 = 8 banks (the full file).
        # ppA needs 3 bufs: the f phase has FW1(i+1) and FW2(i) in flight
        # at once. ppR can be single-buffered because V-dot packs NVP nodes
        # per tile.
        ppA = ctx.enter_context(tc.tile_pool(name="ppA", bufs=3, space="PSUM"))
        ppB = ctx.enter_context(tc.tile_pool(name="ppB", bufs=1, space="PSUM"))
        ppR = ctx.enter_context(tc.tile_pool(name="ppR", bufs=1, space="PSUM"))

        def load_const(ap_dram, shape):
            t = const.tile(list(shape), ap_dram.dtype,
                           tag=f"c_{ap_dram.tensor.name}")
            nc.sync.dma_start(t[:, :], ap_dram)
            return t

        gw2_s = load_const(gw2_d, (A, A))
        fw2_s = load_const(fw2_d, (A, A))
        gw3p_s = load_const(gw3p_d, (A, NCH * A))
        fw1p_s = load_const(fw1p_d, (A, NCH * A))
        bd_s = load_const(bd_d, (A, A))
        u_s = load_const(u_d, (A, N))
        c1_s = load_const(c1_d, (A, N))
        c2_s = load_const(c2_d, (A, N))
        gb2_s = load_const(gb2_d, (A, 1))
        fb2_s = load_const(fb2_d, (A, 1))
        vp2_s = load_const(vp_d, (A, N * NCH * D))

        # Xe[(c,d), (j,t)] = X_emb[d, j, c*CH+t]
        xe = xep.tile([A, N * CH], bf16, tag="xe")

        # ---------------- g phase ----------------
        # Two-deep software pipeline. At iteration j the engines see only
        # ready work at their queue heads: the h1 "front" (Act lrelu, DVE z,
        # Pool min) is emitted two nodes ahead, its finishing adds one node
        # ahead, so nothing upstream of node j's matmuls ever waits.
        xbc_tiles = {}
        h1_front = {}
        h1_tiles = {}

        def emit_xbc(j):
            xbc = workp.tile([A, BC], bf16, tag="xbc", bufs=4)
            nc.sync.dma_start(
                xbc[:, :], xt_d[j : j + 1, :].partition_broadcast(A)
            )
            xbc_tiles[j] = xbc

        def emit_h1_front(j):
            xbc = xbc_tiles.pop(j)
            h1 = workp.tile([A, BC], bf16, tag="h1", bufs=3)
            nc.scalar.activation(
                h1[:, :H1A], xbc[:, :H1A], LRELU,
                bias=c1_s[:, j : j + 1], scale=u_s[:, j : j + 1], alpha=ALPHA,
            )
            # DVE: z = x*u + c1 (bf16). Pool (no PSUM access, no float max):
            # n = (alpha-1)*min(z,0). lrelu = z + n, add split DVE/Pool.
            zt = scrp.tile([A, H1T], bf16, tag="zt", bufs=3)
            mt = scrp.tile([A, H1T], bf16, tag="mt", bufs=3)
            nc.vector.tensor_scalar(zt[:, :], xbc[:, H1A:],
                                    u_s[:, j : j + 1], c1_s[:, j : j + 1],
                                    ALU_MULT, ALU_ADD)
            nc.gpsimd.tensor_scalar(mt[:, :], zt[:, :], 0.0, ALPHA - 1.0,
                                    ALU_MIN, ALU_MULT)
            h1_front[j] = (h1, zt, mt)

        def emit_h1_finish(j):
            h1, zt, mt = h1_front.pop(j)
            nc.vector.tensor_tensor(h1[:, H1A : H1A + H1PD], zt[:, :H1PD],
                                    mt[:, :H1PD], ALU_ADD)
            nc.gpsimd.tensor_tensor(h1[:, H1A + H1PD :], zt[:, H1PD:],
                                    mt[:, H1PD:], ALU_ADD)
            h1_tiles[j] = h1

        emit_xbc(0)
        emit_xbc(1)
        emit_xbc(2)
        emit_h1_front(0)
        emit_h1_front(1)
        emit_h1_finish(0)
        pm3_prev = {}
        for j in range(N):
            # xe copy for node j-1 first: it's a ready op for DVE's queue
            # head and frees pm3's single psum buffer before GW3P(j) asks.
            if j > 0:
                nc.vector.tensor_copy(xe[:, (j - 1) * CH : j * CH],
                                      pm3_prev.pop(j - 1)[:, :])
            # DMA three ahead, h1 front two ahead, finishing adds one ahead:
            # every queued op's inputs were produced at least one full
            # iteration earlier.
            if j + 3 < N:
                emit_xbc(j + 3)
            if j + 2 < N:
                emit_h1_front(j + 2)
            if j + 1 < N:
                emit_h1_finish(j + 1)
            h1 = h1_tiles.pop(j)
            t2 = workp.tile([A, BC], bf16, tag="t2")
            for h in range(2):  # halves of 1024 cols
                pa = ppA.tile([A, HCH], f32, tag="pA")
                for q in range(2):
                    sl = slice(h * HCH + q * CH, h * HCH + (q + 1) * CH)
                    nc.tensor.matmul(
                        pa[:, q * CH : (q + 1) * CH], gw2_s[:, :],
                        h1[:, sl], start=True, stop=True,
                    )
                off = h * HCH
                if T2AH:
                    nc.scalar.activation(
                        t2[:, off : off + T2AH], pa[:, :T2AH], LRELU,
                        bias=gb2_s[:, 0:1], alpha=ALPHA,
                    )
                pcols = HCH - T2AH
                if pcols:
                    # PSUM-sourced tail: DVE-only pair (GPSIMD can't read
                    # PSUM).
                    m2 = scrp.tile([A, pcols], bf16, tag="m2")
                    if zero_b2:
                        nc.vector.tensor_scalar(
                            m2[:, :], pa[:, T2AH:], ALPHA, None, ALU_MULT)
                        nc.vector.tensor_tensor(
                            t2[:, off + T2AH : off + HCH], pa[:, T2AH:],
                            m2[:, :], ALU_MAX)
                    else:
                        z2 = scrp.tile([A, pcols], bf16, tag="z2")
                        nc.vector.tensor_scalar(
                            z2[:, :], pa[:, T2AH:], gb2_s[:, 0:1], None,
                            ALU_ADD)
                        nc.gpsimd.tensor_scalar(
                            m2[:, :], z2[:, :], 0.0, ALPHA - 1.0,
                            ALU_MIN, ALU_MULT)
                        nc.gpsimd.tensor_tensor(
                            t2[:, off + T2AH : off + HCH], z2[:, :],
                            m2[:, :], ALU_ADD)
            # X_emb = g_W3^T @ (H1 + lrelu(.)) via 8 accumulating matmuls,
            # chunk c routed to psum rows 32c by the padded stationary.
            # h1 passes first: they are ready before t2 lands. The copy to
            # xe is deferred to the top of the next iteration.
            pm3 = ppB.tile([A, CH], f32, tag="pB")
            for c in range(NCH):
                nc.tensor.matmul(pm3[:, :], gw3p_s[:, c * A : (c + 1) * A],
                                 h1[:, c * CH : (c + 1) * CH],
                                 start=(c == 0), stop=False)
            for c in range(NCH):
                nc.tensor.matmul(pm3[:, :], gw3p_s[:, c * A : (c + 1) * A],
                                 t2[:, c * CH : (c + 1) * CH],
                                 start=False, stop=(c == NCH - 1))
            pm3_prev[j] = pm3
        nc.vector.tensor_copy(xe[:, (N - 1) * CH :],
                              pm3_prev.pop(N - 1)[:, :])

        # ---------------- aggregation ----------------
        # T1: Xe[(c,d),(j,t)] -> Xt[(c,j),(t,d)]
        xt3 = xe.rearrange("p (j t) -> p j t", j=N).transpose([0, 2, 1])
        xtile = xep.tile([A, CH * D], bf16, tag="xt")
        xto = xtile.rearrange("p (t d) -> p t d", d=D)
        TS = 8  # split into 8 ops for overlap
        tstep = CH // TS
        for s in range(TS):
            nc.vector.transpose(
                xto[:, s * tstep : (s + 1) * tstep, :],
                xt3[:, s * tstep : (s + 1) * tstep, :],
            )
        # agg windows + T2-back: psum[(c,i),(t16,d)] -> Xa[(c,d),(i,t)].
        # StreamTranspose can't convert dtypes: DVE transposes psum f32 into
        # a small rolling f32 buffer, then the (agg-idle) Pool engine does a
        # strided convert-copy into bf16 Xa.
        xa = xep.tile([A, N * CH], bf16, tag="xa")
        xa3 = xa.rearrange("p (i t) -> p i t", i=N).transpose([0, 2, 1])
        WT = HCH // D  # 32 t per window
        for w in range(CH // WT):  # 16 windows of 1024 cols
            pg = ppA.tile([A, HCH], f32, tag="pA", name=f"pg_{w}")
            for q in range(2):  # psum bank per matmul output
                nc.tensor.matmul(
                    pg[:, q * CH : (q + 1) * CH], bd_s[:, :],
                    xtile[:, w * HCH + q * CH : w * HCH + (q + 1) * CH],
                    start=True, stop=True,
                )
            xat = scrp.tile([A, HCH], f32, tag="xat", bufs=3)
            nc.vector.transpose(
                xat.rearrange("p (t i) -> p t i", i=D)[:, :, :],
                pg.rearrange("p (t d) -> p t d", d=D)[:, :, :],
            )
            nc.gpsimd.tensor_copy(
                xa3[:, w * WT : (w + 1) * WT, :],
                xat.rearrange("p (t i) -> p t i", i=D)[:, :, :],
            )

        # ---------------- f phase ----------------
        # Pipelined at HALF-node granularity (64 units u = 2*i + h): the f
        # dependency chain factors by halves (FW2 of half h reads only hf1
        # of half h; V-dot chunk c sits in one half), so with [A, 1024]
        # psum tiles and ppA bufs=3, three half-units can be in flight
        # where full nodes could not. Stages: S1 = FW1 + hf1 act,
        # S2 = FW2 + tf lrelu, S3 = hf2 add (Pool, half 0) + V-dot.
        # V-dot psum packing: NVP nodes share one [32, CH] psum tile; the
        # VP2 stationary routes node i chunk c to psum row 4*(i%NVP)+c.
        hf1_tiles = {}
        tf_half = {}
        vdot_state = {"pr": None}

        def emit_fs1(u):
            i, h = divmod(u, 2)
            if h == 0:
                hf1 = workp.tile([A, BC], bf16, tag="hf1", bufs=4,
                                 name=f"hf1_{i}")
                hf1_tiles[i] = hf1
            hf1 = hf1_tiles[i]
            rhs = xa[:, i * CH : (i + 1) * CH]
            pa = ppA.tile([A, HCH], f32, tag="pA", name=f"pf1_{u}")
            for q in range(2):
                c = h * 2 + q
                nc.tensor.matmul(
                    pa[:, q * CH : (q + 1) * CH],
                    fw1p_s[:, c * A : (c + 1) * A], rhs,
                    start=True, stop=True,
                )
            nc.scalar.activation(
                hf1[:, h * HCH : (h + 1) * HCH], pa[:, :], LRELU,
                bias=c2_s[:, i : i + 1], alpha=ALPHA,
            )

        def emit_fs2(u):
            i, h = divmod(u, 2)
            hf1 = hf1_tiles[i]
            tf = workp.tile([A, HCH], bf16, tag="tf", bufs=3,
                            name=f"tf_{u}")
            pa = ppA.tile([A, HCH], f32, tag="pA", name=f"pf2_{u}")
            for q in range(2):
                c = h * 2 + q
                nc.tensor.matmul(
                    pa[:, q * CH : (q + 1) * CH], fw2_s[:, :],
                    hf1[:, c * CH : (c + 1) * CH], start=True, stop=True,
                )
            if TFAH:
                nc.scalar.activation(
                    tf[:, :TFAH], pa[:, :TFAH], LRELU,
                    bias=fb2_s[:, 0:1], alpha=ALPHA,
                )
            pcols = HCH - TFAH
            if pcols:
                # PSUM-sourced tail: DVE-only pair (GPSIMD can't read PSUM).
                mf = scrp.tile([A, pcols], bf16, tag="mf")
                if zero_b2:
                    nc.vector.tensor_scalar(
                        mf[:, :], pa[:, TFAH:], ALPHA, None, ALU_MULT)
                    nc.vector.tensor_tensor(
                        tf[:, TFAH:], pa[:, TFAH:], mf[:, :], ALU_MAX)
                else:
                    zf = scrp.tile([A, pcols], bf16, tag="zf")
                    nc.vector.tensor_scalar(
                        zf[:, :], pa[:, TFAH:], fb2_s[:, 0:1], None,
                        ALU_ADD)
                    nc.gpsimd.tensor_scalar(
                        mf[:, :], zf[:, :], 0.0, ALPHA - 1.0,
                        ALU_MIN, ALU_MULT)
                    nc.gpsimd.tensor_tensor(
                        tf[:, TFAH:], zf[:, :], mf[:, :], ALU_ADD)
            tf_half[u] = tf

        def emit_fs3(u):
            i, h = divmod(u, 2)
            hf1 = hf1_tiles[i] if h == 0 else hf1_tiles.pop(i)
            tf = tf_half.pop(u)
            if u % (2 * NVP) == 0:
                vdot_state["pr"] = ppR.tile([D, CH], f32, tag="pR",
                                            name=f"pr_{u}")
            pr = vdot_state["pr"]
            for q in range(2):
                c = h * 2 + q
                base = (i * NCH + c) * D
                st = (i % NVP == 0 and c == 0)
                sp = (i % NVP == NVP - 1 and c == NCH - 1)
                if h == 0:
                    # Pool (idle in f) materializes hf2 for half 0
                    hp = scrp.tile([A, CH], bf16, tag="hf2c",
                                   name=f"hf2_{u}_{q}")
                    nc.gpsimd.tensor_tensor(
                        hp[:, :], hf1[:, c * CH : (c + 1) * CH],
                        tf[:, q * CH : (q + 1) * CH], ALU_ADD)
                    nc.tensor.matmul(pr[:, :], vp2_s[:, base : base + D],
                                     hp[:, :], start=st, stop=sp)
                else:
                    nc.tensor.matmul(pr[:, :], vp2_s[:, base : base + D],
                                     hf1[:, c * CH : (c + 1) * CH],
                                     start=st, stop=False)
                    nc.tensor.matmul(pr[:, :], vp2_s[:, base : base + D],
                                     tf[:, q * CH : (q + 1) * CH],
                                     start=False, stop=sp)
            if u % (2 * NVP) == 2 * NVP - 1:
                g0 = i - (NVP - 1)
                osb = outp.tile([NVP * NCH, CH], f32, tag="o")
                nc.vector.tensor_copy(osb[:, :], pr[: NVP * NCH, :])
                nc.sync.dma_start(
                    out_d[g0 : g0 + NVP, :].rearrange(
                        "o (c t) -> (o c) t", c=NCH),
                    osb[:, :],
                )

        NU = 2 * N
        emit_fs1(0)
        emit_fs2(0)
        emit_fs1(1)
        for u in range(NU):
            if u + 2 < NU:
                emit_fs1(u + 2)
            if u + 1 < NU:
                emit_fs2(u + 1)
            emit_fs3(u)

    nc.compile()
    return nc


_NC_CACHE = {}


def _get_program(zero_b2=True):
    if zero_b2 not in _NC_CACHE:
        _NC_CACHE[zero_b2] = _build_program(zero_b2)
    return _NC_CACHE[zero_b2]


def _bf16(x):
    import ml_dtypes
    return np.asarray(x, np.float32).astype(ml_dtypes.bfloat16)


def _host_consts(W, embeddings, g_W1, g_b1, g_W2, g_b2, g_W3, g_b3,
                 f_W1, f_b1, f_W2, f_b2, f_W3, f_b3):
    f = np.float32
    W_adj = (W * (1.0 - np.eye(N, dtype=f))).astype(f)
    U = np.ascontiguousarray(g_W1[:D].T, dtype=f)                    # [A, N]
    C1 = np.ascontiguousarray((embeddings @ g_W1[D:] + g_b1).T, f)   # [A, N]
    s = W_adj.sum(axis=0)                                            # [N]
    C2 = (embeddings @ f_W1[D:] + f_b1 + np.outer(s, g_b3 @ f_W1[:D]))
    C2 = np.ascontiguousarray(C2.T, dtype=f)                         # [A, N]
    GW3P = np.zeros((A, NCH * A), f)
    FW1P = np.zeros((A, NCH * A), f)
    for c in range(NCH):
        GW3P[:, c * A + c * D : c * A + (c + 1) * D] = g_W3
        FW1P[c * D : (c + 1) * D, c * A : (c + 1) * A] = f_W1[:D]
    BD = np.kron(np.eye(NCH, dtype=f), W_adj).astype(f)
    VP2 = np.zeros((A, N * NCH * D), f)
    for i in range(N):
        for c in range(NCH):
            VP2[:, (i * NCH + c) * D + NCH * (i % NVP) + c] = f_W3[:, i]
    return {
        "GW2": _bf16(g_W2),
        "FW2": _bf16(f_W2),
        "GW3P": _bf16(GW3P), "FW1P": _bf16(FW1P), "BD": _bf16(BD),
        "U": U, "C1": C1, "C2": C2,
        "GB2": np.ascontiguousarray(g_b2.reshape(A, 1), f),
        "FB2": np.ascontiguousarray(f_b2.reshape(A, 1), f),
        "VP2": _bf16(VP2),
    }


def _kernel_numpy(X, W, embeddings, g_W1, g_b1, g_W2, g_b2, g_W3, g_b3,
                  f_W1, f_b1, f_W2, f_b2, f_W3, f_b3, group_mask):
    # general fallback (non-identity group_mask)
    def lrelu(x):
        return np.where(x > 0, x, ALPHA * x)

    def mlp(x, W1, b1, W2, b2, W3, b3):
        h = lrelu(x @ W1 + b1)
        h = h + lrelu(h @ W2 + b2)
        return h @ W3 + b3

    n = W.shape[0]
    W_adj = W * (1.0 - np.eye(n, dtype=W.dtype))
    Xm = X[:, None, :] * group_mask
    E = np.broadcast_to(embeddings, (X.shape[0], n, embeddings.shape[1]))
    Xe = mlp(np.concatenate([Xm, E], 2), g_W1, g_b1, g_W2, g_b2, g_W3, g_b3)
    Xa = np.einsum("ji,bjd->bid", W_adj, Xe)
    Xr = mlp(np.concatenate([Xa, E], 2), f_W1, f_b1, f_W2, f_b2, f_W3, f_b3)
    return (Xr * group_mask).sum(axis=1).astype(np.float32)


def kernel(X, W, embeddings, g_W1, g_b1, g_W2, g_b2, g_W3, g_b3,
           f_W1, f_b1, f_W2, f_b2, f_W3, f_b3, group_mask, _run_kw=None):
    if not np.allclose(group_mask, np.eye(N, D, dtype=np.float32)):
        return _kernel_numpy(X, W, embeddings, g_W1, g_b1, g_W2, g_b2, g_W3,
                             g_b3, f_W1, f_b1, f_W2, f_b2, f_W3, f_b3,
                             group_mask)

    from concourse import bass_utils

    zero_b2 = not (np.any(g_b2) or np.any(f_b2))
    consts = _host_consts(W, embeddings, g_W1, g_b1, g_W2, g_b2, g_W3, g_b3,
                          f_W1, f_b1, f_W2, f_b2, f_W3, f_b3)
    XT = _bf16(np.asarray(X, np.float32).T)  # [N, B] bf16
    in_maps = []
    for k in range(NCORES):
        m = dict(consts)
        m["XT"] = np.ascontiguousarray(XT[:, k * BC : (k + 1) * BC])
        in_maps.append(m)

    nc = _get_program(zero_b2)
    res = bass_utils.run_bass_kernel_spmd(
        nc, in_maps, core_ids=list(range(NCORES)), **(_run_kw or {})
    )
    out = np.empty((B, D), np.float32)
    for k in range(NCORES):
        out[k * BC : (k + 1) * BC, :] = res.results[k]["OUT"].T
    out += f_b3.reshape(1, D).astype(np.float32)
    if _run_kw:
        kernel.last_results = res
    return out
